# revision 14
# baseline (speedup 1.0000x reference)
"""CapsNet class-capsule dynamic routing kernel for 8x Trainium2 NeuronCores.

Problem: B=256, N_IN=1152, D_IN=8, N_CLS=10, D_OUT=16, 3 routing iters.
Sharding: data-parallel over batch (32 samples/core), W replicated.

v2 design (per core, BL=32, i = sg*16 + r*4 + i4):
  u_hat phase: per (sg, r) K=32 block-diag matmuls (4-way row tiling) as
  before, PLUS s0 = 0.1*sum_i u_hat computed directly as 72 accumulating
  K=128 matmuls xT[(r,i4,d), b] @ W[(r,i4,d), (o,c)] interleaved on PE —
  kills iter0's select-matmul pass. PSUM->SBUF u_hat copies split across
  DVE/Act/GpSimd by o-columns.
  Routing iters: per-chunk fused pipeline (agr mul+tree -> bb -> exp ->
  z/rz -> cw -> s-mul -> per-sg select-matmuls accumulating one [32,160]
  PSUM region, no sum3). Three agr chunks + one s chunk on GpSimd to
  offload DVE. v replication via PE matmul with a replicator lhsT instead
  of 4 DMAs. Squash uses a DVE rsqrt bit-hack (no Act table switches; Act
  only ever runs Exp/Copy/Square from one table).
"""

import numpy as np

B, N_IN, D_IN, N_CLS, D_OUT = 256, 1152, 8, 10, 16
NCORES = 8
BL = B // NCORES          # 32
SG = N_IN // 16           # 72 supergroups
CH = 6                    # supergroups per routing chunk
NCH = SG // CH            # 12 chunks
CO = D_OUT * N_CLS        # 160

AGR_GPS = (0, 1, 2)       # agreement chunks computed on gpsimd
SMUL_GPS = (0,)           # s-pass mul chunks computed on gpsimd
# DVE-owned chunks first so gpsimd's (0-2) overlap them; grouped by 3 for
# the softmax z/rz ops.
ZGROUPS = ((3, 4, 5), (6, 7, 8), (9, 10, 11), (0, 1, 2))

_CACHE = {}


def _build_program(loop_n=None, stop_after=None):
    from contextlib import ExitStack

    import concourse.tile as tile
    from concourse import bacc, mybir

    f16 = mybir.dt.float16
    f32 = mybir.dt.float32
    u32 = mybir.dt.uint32
    AX = mybir.AxisListType
    OP = mybir.AluOpType
    ACTF = mybir.ActivationFunctionType

    nc = bacc.Bacc("TRN2", target_bir_lowering=False, debug=False, num_devices=1)

    xs_d = nc.dram_tensor("xs", [4, 32, SG, 128], f16, kind="ExternalInput")
    ws_d = nc.dram_tensor("ws", [128, SG, CO], f16, kind="ExternalInput")
    xt_d = nc.dram_tensor("xt", [128, SG, BL], f16, kind="ExternalInput")
    sel_d = nc.dram_tensor("sel", [128, BL], f16, kind="ExternalInput")
    rep_d = nc.dram_tensor("rep", [BL, 128], f16, kind="ExternalInput")
    v_d = nc.dram_tensor("v", [BL, D_OUT, N_CLS], f32, kind="ExternalOutput")

    with tile.TileContext(nc) as tc, ExitStack() as ctx:
        persist = ctx.enter_context(tc.tile_pool(name="persist", bufs=1))
        wpool = ctx.enter_context(tc.tile_pool(name="wpool", bufs=12))
        scratch = ctx.enter_context(tc.tile_pool(name="scratch", bufs=3))
        small = ctx.enter_context(tc.tile_pool(name="small", bufs=2))

        # ---- persistent tiles ----
        x_sb = persist.tile([128, SG, 128], f16)
        xt_sb = persist.tile([128, SG, BL], f16)
        sel_sb = persist.tile([128, BL], f16)
        rep_sb = persist.tile([BL, 128], f16)
        u_hat = persist.tile([128, SG, 4, D_OUT, N_CLS], f16)
        bb = persist.tile([128, SG, 4, N_CLS], f16)
        e = persist.tile([128, SG, 4, N_CLS], f16)
        v_exp = persist.tile([128, D_OUT, N_CLS], f16)

        # ---- pre-loop loads ----
        for r in range(4):
            nc.sync.dma_start(x_sb[32 * r:32 * r + 32], xs_d.ap()[r])
        nc.sync.dma_start(xt_sb[:], xt_d.ap())
        nc.sync.dma_start(sel_sb[:], sel_d.ap())
        nc.sync.dma_start(rep_sb[:], rep_d.ap())

        loop_cm = tc.For_i(0, loop_n, 1) if loop_n else None
        if loop_cm is not None:
            loop_cm.__enter__()

        # ---- u_hat phase (baseline-safe PSUM pattern: per-r banks, one-shot
        # groups). W chunks stay resident so the s0 tail can reuse them. ----
        uctx = ExitStack()
        upsum_pool = uctx.enter_context(tc.tile_pool(name="ups", bufs=2, space="PSUM"))
        wbufs = []
        for wc in range(NCH):
            wbuf = wpool.tile([128, CH, CO], f16, name="wbuf")
            wbufs.append(wbuf)
            nc.sync.dma_start(wbuf[:], ws_d.ap()[:, wc * CH:(wc + 1) * CH])
            for s8 in range(CH):
                sg = wc * CH + s8
                ups = upsum_pool.tile([128, 4, 512], f32, name="ups")
                for r in range(4):
                    nc.tensor.matmul(
                        ups[:, r, 0:CO],
                        x_sb[32 * r:32 * r + 32, sg, :],
                        wbuf[32 * r:32 * r + 32, s8, :],
                        start=True, stop=True,
                        tile_position=(32 * r, 0),
                    )
                usrc = ups[:, :, 0:CO].rearrange("p r (o c) -> p r o c", o=D_OUT)
                if sg % 2 == 0:
                    nc.vector.tensor_copy(u_hat[:, sg], usrc)
                else:
                    nc.scalar.copy(u_hat[:, sg], usrc)

        uctx.close()

        # ---- s0 = 0.1*sum_i u_hat via 72 accumulating K=128 matmuls
        # (single consecutive chain, one bank, one tile position). ----
        uctx = ExitStack()
        s0_pool = uctx.enter_context(tc.tile_pool(name="s0p", bufs=1, space="PSUM"))
        s0_ps = s0_pool.tile([BL, 512], f32, name="s0_ps")
        for sg in range(SG):
            nc.tensor.matmul(
                s0_ps[:, 0:CO], xt_sb[:, sg, :], wbufs[sg // CH][:, sg % CH, :],
                start=(sg == 0), stop=(sg == SG - 1),
            )
        s0_sb = small.tile([BL, CO], f32)
        nc.vector.tensor_copy(s0_sb[:], s0_ps[:, 0:CO])
        uctx.close()

        spsum_pool = ctx.enter_context(tc.tile_pool(name="sps", bufs=2, space="PSUM"))
        vrep_pool = ctx.enter_context(tc.tile_pool(name="vps", bufs=2, space="PSUM"))

        def squash(s_in, want_f32, sq_via_act):
            """s_in: [32, CO] AP (SBUF f32 or PSUM f32). Returns (v_sb, v32)."""
            s2 = small.tile([BL, CO], f32)
            if sq_via_act:
                nc.scalar.square(s2[:], s_in)
            else:
                nc.vector.tensor_mul(s2[:], s_in, s_in)
            sq = small.tile([BL, N_CLS], f32)
            nc.vector.tensor_reduce(
                sq[:], s2[:].rearrange("p (o c) -> p c o", o=D_OUT),
                axis=AX.X, op=OP.add)
            # sc = sq/((1+sq)*(sqrt(sq)+eps))
            t = small.tile([BL, N_CLS], f32)
            nc.scalar.sqrt(t[:], sq[:])
            nc.vector.tensor_scalar(t[:], t[:], 1e-8, None, OP.add)
            q1 = small.tile([BL, N_CLS], f32)
            nc.vector.tensor_scalar(q1[:], sq[:], 1.0, None, OP.add)
            den = small.tile([BL, N_CLS], f32)
            nc.vector.tensor_mul(den[:], q1[:], t[:])
            rden = small.tile([BL, N_CLS], f32)
            nc.vector.reciprocal(rden[:], den[:])
            sc = small.tile([BL, N_CLS], f32)
            nc.vector.tensor_mul(sc[:], sq[:], rden[:])
            sc_b = sc[:].unsqueeze(1).to_broadcast([BL, D_OUT, N_CLS])
            s_v = s_in.rearrange("p (o c) -> p o c", o=D_OUT)
            v_sb = small.tile([BL, D_OUT, N_CLS], f16)
            nc.vector.tensor_mul(v_sb[:], s_v, sc_b)
            v32 = None
            if want_f32:
                v32 = small.tile([BL, D_OUT, N_CLS], f32)
                nc.vector.tensor_mul(v32[:], s_v, sc_b)
            return v_sb, v32

        def vrep(v_sb):
            """Replicate v_sb [32,(o,c)] to all 128 partitions via PE."""
            vps = vrep_pool.tile([128, 512], f32, name="vps")
            nc.tensor.matmul(
                vps[:, 0:CO], rep_sb[:],
                v_sb[:].rearrange("p o c -> p (o c)"),
                start=True, stop=True)
            nc.scalar.copy(
                v_exp[:],
                vps[:, 0:CO].rearrange("p (o c) -> p o c", o=D_OUT))

        if stop_after in ("uhat", "uhat_nos0"):
            vdump = small.tile([BL, D_OUT, N_CLS], f32)
            nc.vector.tensor_copy(vdump[:], u_hat[0:32, 0, 0])
            nc.sync.dma_start(v_d.ap(), vdump[:])
        if stop_after in ("s0", "s0skip"):
            nc.sync.dma_start(
                v_d.ap(), s0_sb[:].rearrange("p (o c) -> p o c", o=D_OUT))

        iters = ()
        if stop_after is None:
            iters = (1, 2)
        elif stop_after == "iter1":
            iters = (1,)

        if stop_after not in ("uhat", "uhat_nos0", "s0", "s0skip"):
            v_sb, v32 = squash(s0_sb[:], stop_after == "iter0", False)
            if stop_after == "iter0":
                nc.sync.dma_start(v_d.ap(), v32[:])
            vrep(v_sb)

        for it in iters:
            # ---- fused agreement -> softmax -> s pipeline, chunked ----
            def agr_chunk(chk):
                eng = nc.gpsimd if chk in AGR_GPS else nc.vector
                sl = slice(chk * CH, (chk + 1) * CH)
                u_ch = u_hat[:, sl]
                v_bb = (v_exp[:].unsqueeze(1).unsqueeze(1)
                        .to_broadcast([128, CH, 4, D_OUT, N_CLS]))
                prod = scratch.tile([128, CH, 4, D_OUT, N_CLS], f16, name="prod")
                eng.tensor_mul(prod[:], u_ch, v_bb)
                eng.tensor_add(prod[:, :, :, 0:8], prod[:, :, :, 0:8],
                               prod[:, :, :, 8:16])
                eng.tensor_add(prod[:, :, :, 0:4], prod[:, :, :, 0:4],
                               prod[:, :, :, 4:8])
                eng.tensor_add(prod[:, :, :, 0:2], prod[:, :, :, 0:2],
                               prod[:, :, :, 2:4])
                if it == 1:
                    eng.tensor_add(bb[:, sl], prod[:, :, :, 0], prod[:, :, :, 1])
                else:
                    eng.tensor_add(prod[:, :, :, 0], prod[:, :, :, 0],
                                   prod[:, :, :, 1])
                    eng.tensor_add(bb[:, sl], bb[:, sl], prod[:, :, :, 0])
                # softmax numerator (logits tiny: no max-subtraction)
                nc.scalar.activation(e[:, sl], bb[:, sl], ACTF.Exp)

            def zgroup(grp):
                lo, hi = grp[0] * CH, (grp[-1] + 1) * CH
                n = hi - lo
                z32 = small.tile([128, 3 * CH * 4], f32, name="z32", tag="z32")
                nc.vector.tensor_reduce(
                    z32[:, 0:n * 4],
                    e[:, lo:hi].rearrange("p s r c -> p (s r) c"),
                    axis=AX.X, op=OP.add)
                rz32 = small.tile([128, 3 * CH * 4], f32, name="rz32", tag="rz32")
                nc.vector.reciprocal(rz32[:, 0:n * 4], z32[:, 0:n * 4])
                rz = small.tile([128, 3 * CH * 4], f16, name="rz", tag="rz")
                nc.vector.tensor_copy(rz[:, 0:n * 4], rz32[:, 0:n * 4])
                rz_b = (rz[:, 0:n * 4].rearrange("p (s r) -> p s r", s=n)
                        .unsqueeze(-1).to_broadcast([128, n, 4, N_CLS]))
                nc.vector.tensor_mul(e[:, lo:hi], e[:, lo:hi], rz_b)

            def smul_chunk(chk, k):
                eng = nc.gpsimd if chk in SMUL_GPS else nc.vector
                sl = slice(chk * CH, (chk + 1) * CH)
                cw_b = (e[:, sl].unsqueeze(3)
                        .to_broadcast([128, CH, 4, D_OUT, N_CLS]))
                prod2 = scratch.tile([128, CH, 4, D_OUT, N_CLS], f16, name="prod2")
                eng.tensor_mul(prod2[:], u_hat[:, sl], cw_b)
                for s8 in range(CH):
                    for r in range(4):
                        nc.tensor.matmul(
                            s_ps[0:32, 0:CO], sel_sb[:, 0:32],
                            prod2[:, s8, r],
                            start=(k == 0), stop=(k == n_mm - 1))
                        k += 1
                return k

            s_ps = spsum_pool.tile([32, 512], f32, name="s_ps")
            n_mm = NCH * CH * 4
            k = 0
            for gi, grp in enumerate(ZGROUPS):
                for chk in grp:
                    agr_chunk(chk)
                zgroup(grp)
                for chk in grp:
                    k = smul_chunk(chk, k)

            last = it == 2 or stop_after == "iter1"
            v_sb, v32 = squash(s_ps[0:32, 0:CO], last, True)
            if last:
                nc.sync.dma_start(v_d.ap(), v32[:])
            else:
                vrep(v_sb)

        if loop_cm is not None:
            loop_cm.__exit__(None, None, None)

    nc.compile()
    return nc


def _get_program(loop_n=None, stop_after=None):
    key = ("nc", loop_n, stop_after)
    if key not in _CACHE:
        _CACHE[key] = _build_program(loop_n, stop_after)
    return _CACHE[key]


def _prep_inputs(x, W):
    """Host-side layout prep. Returns per-core input maps."""
    sel = (np.arange(128)[:, None] % 32 == np.arange(BL)[None, :]).astype(np.float16)
    rep = np.ascontiguousarray(sel.T)
    # i = sg*16 + r*4 + i4 ; u_hat lhsT for (sg, r) is [32=(i4,d), 128=(i4,b)]
    # block-diagonal of xT; rhs is W stacked [128=(r,i4,d), sg, (o,c)].
    Wr = np.asarray(W[0]).reshape(SG, 4, 4, N_CLS, D_OUT, D_IN)  # sg r i4 c o d
    ws = np.ascontiguousarray(
        Wr.transpose(1, 2, 5, 0, 4, 3)                     # r i4 d sg o c
    ).astype(np.float16).reshape(128, SG, CO)
    in_maps = []
    for c in range(NCORES):
        xl = np.asarray(x[c * BL:(c + 1) * BL])            # [32, 1152, 8]
        xr = xl.reshape(BL, SG, 4, 4, D_IN)                # b sg r i4 d
        m = xr.transpose(2, 3, 4, 1, 0).astype(np.float16)  # r i4 d sg b
        xt = np.ascontiguousarray(m * np.float16(1.0 / N_CLS)).reshape(128, SG, BL)
        xbd = np.zeros((4, 4, D_IN, SG, 4, BL), np.float16)
        for q in range(4):
            xbd[:, q, :, :, q, :] = m[:, q]
        xsc = np.ascontiguousarray(xbd).reshape(4, 32, SG, 128)
        in_maps.append({"xs": xsc, "ws": ws, "xt": xt, "sel": sel, "rep": rep})
    return in_maps


def kernel(x, W):
    from concourse.bass_utils import run_bass_kernel_spmd

    nc = _get_program()
    in_maps = _prep_inputs(x, W)
    res = run_bass_kernel_spmd(nc, in_maps, core_ids=list(range(NCORES)))
    outs = []
    for c in range(NCORES):
        v = res.results[c]["v"]                  # [32, 16, 10]
        outs.append(v.transpose(0, 2, 1))        # [32, 10, 16]
    return np.ascontiguousarray(np.concatenate(outs, axis=0)).astype(np.float32)
